# revision 5
# baseline (speedup 1.0000x reference)
"""GRU-D decoder kernel for Trainium2 (8 NeuronCores, data-parallel over batch).

Math (mask == ones everywhere, which the reference hardcodes):
  x_hat = C (constant), d = dt broadcast, gamma_x unused.
  gamma[t,b,j] = exp(-relu(dt[t,b] * colsum(Wgh)[j] + bgh[j]))   (precomputed host-side)
  per step: hdec = gamma_t * h
            z = sigmoid(hdec @ Wz_h + Az0);  r = sigmoid(hdec @ Wr_h + Ar0)
            htl = tanh((r*hdec) @ Wh_h + Ah0)
            h = hdec + z*(htl - hdec)
  out[t] = h_t @ Wlin + blin
  where A?0 = C @ W?_x + colsum(W?_m) + b?  (time-constant, precomputed host-side).

Device layout: everything transposed (H on partitions as 4 tiles of 128,
batch=64 on the free dim), packed as SBUF tiles (128, 4*64) with column
index = kt*64 + b.  Gate matmuls use the weight blocks as stationary
operands and hdec slices as moving operands; outputs land natively in the
same transposed layout, so no transposes are needed anywhere.
"""

import numpy as np
import ml_dtypes

T, B, H, O = 100, 512, 512, 512
NCORES = 8
BL = B // NCORES  # 64
KC = 4  # contraction chunks of 128
JT = 4  # output j-tiles of 128
GCH = 20  # gamma chunk (steps per DMA)

_BUILD_CACHE = {}


def _build_program():
    if "nc" in _BUILD_CACHE:
        return _BUILD_CACHE["nc"]

    import concourse.bass as bass
    import concourse.tile as tile
    import concourse.mybir as mybir
    from concourse import bacc
    from contextlib import ExitStack

    f32 = mybir.dt.float32
    bf16 = mybir.dt.bfloat16
    AF = mybir.ActivationFunctionType

    nc = bacc.Bacc("TRN2", target_bir_lowering=False, debug=False,
                   num_devices=NCORES)

    gam_d = nc.dram_tensor("gam", [128, T, JT * BL], f32, kind="ExternalInput")
    wzr_d = nc.dram_tensor("wzr", [128, KC * 2 * JT * 128], bf16, kind="ExternalInput")
    wht_d = nc.dram_tensor("wht", [128, KC * JT * 128], bf16, kind="ExternalInput")
    wlin_d = nc.dram_tensor("wlin", [128, KC * O], bf16, kind="ExternalInput")
    a0z_d = nc.dram_tensor("a0z", [128, JT * BL], bf16, kind="ExternalInput")
    a0r_d = nc.dram_tensor("a0r", [128, JT * BL], bf16, kind="ExternalInput")
    a0h_d = nc.dram_tensor("a0h", [128, JT * BL], bf16, kind="ExternalInput")
    ident_d = nc.dram_tensor("ident", [128, 128], bf16, kind="ExternalInput")
    blin_d = nc.dram_tensor("blinb", [BL, O], f32, kind="ExternalInput")
    out_d = nc.dram_tensor("out", [T, BL, O], f32, kind="ExternalOutput")

    FR = JT * BL  # 256 free size of a transposed tile

    with tile.TileContext(nc) as tc, ExitStack() as ctx:
        constp = ctx.enter_context(tc.tile_pool(name="const", bufs=1))
        gpool = ctx.enter_context(tc.tile_pool(name="gam", bufs=2))
        statep = ctx.enter_context(tc.tile_pool(name="state", bufs=1))
        hdp = ctx.enter_context(tc.tile_pool(name="hd", bufs=2))
        actp = ctx.enter_context(tc.tile_pool(name="act", bufs=2))
        outp = ctx.enter_context(tc.tile_pool(name="outs", bufs=3))
        pzp = ctx.enter_context(tc.tile_pool(name="pz", bufs=2, space="PSUM"))
        prp = ctx.enter_context(tc.tile_pool(name="pr", bufs=2, space="PSUM"))
        php = ctx.enter_context(tc.tile_pool(name="ph", bufs=2, space="PSUM"))
        pjp = ctx.enter_context(tc.tile_pool(name="pj", bufs=2, space="PSUM"))

        wzr = constp.tile([128, KC * 2 * JT * 128], bf16)
        nc.sync.dma_start(wzr[:], wzr_d[:])
        wht = constp.tile([128, KC * JT * 128], bf16)
        nc.sync.dma_start(wht[:], wht_d[:])
        wlin = constp.tile([128, KC * O], bf16)
        nc.sync.dma_start(wlin[:], wlin_d[:])
        a0z = constp.tile([128, FR], bf16)
        nc.sync.dma_start(a0z[:], a0z_d[:])
        a0r = constp.tile([128, FR], bf16)
        nc.sync.dma_start(a0r[:], a0r_d[:])
        a0h = constp.tile([128, FR], bf16)
        nc.sync.dma_start(a0h[:], a0h_d[:])
        ident = constp.tile([128, 128], bf16)
        nc.sync.dma_start(ident[:], ident_d[:])
        blinb = constp.tile([BL, O], f32)
        nc.sync.dma_start(blinb[:], blin_d[:])

        h = statep.tile([128, FR], f32)
        nc.vector.memset(h[:], 0.0)

        # weight block column offset helpers
        def wzr_blk(g, jo, kc):
            return wzr[:, ((kc * 2 + g) * JT + jo) * 128:((kc * 2 + g) * JT + jo + 1) * 128]

        def wht_blk(jo, kc):
            return wht[:, (kc * JT + jo) * 128:(kc * JT + jo + 1) * 128]

        gt = None
        gt_next = None
        hbf_prev = None
        pj_prev = None

        for t in range(T):
            c, o = divmod(t, GCH)
            if o == 0:
                if t == 0:
                    gt = gpool.tile([128, GCH * FR], f32, tag="gchunk")
                    nc.sync.dma_start(gt[:], gam_d[:, 0:GCH, :])
                else:
                    gt = gt_next
            if o == GCH // 2 and t + GCH <= T - 1:
                t0 = (c + 1) * GCH
                t1 = min(t0 + GCH, T)
                gt_next = gpool.tile([128, GCH * FR], f32, tag="gchunk")
                nc.sync.dma_start(gt_next[:, 0:(t1 - t0) * FR], gam_d[:, t0:t1, :])

            g = gt[:, o * FR:(o + 1) * FR]

            # hdec in bf16 (matmul input) and f32 (blend input)
            hdb = hdp.tile([128, FR], bf16, tag="hdb")
            nc.vector.tensor_mul(hdb[:], g, h[:])
            hdf = hdp.tile([128, FR], f32, tag="hdf")
            nc.vector.tensor_mul(hdf[:], g, h[:])

            # ---- r gate matmuls (emitted first so sigmoid(r) overlaps z matmuls)
            pr = prp.tile([128, FR], f32, tag="pr")
            nc.tensor.matmul(pr[:], ident[:], a0r[:], start=True, stop=False)
            for jo in range(JT):
                for kc in range(KC):
                    nc.tensor.matmul(
                        pr[:, jo * BL:(jo + 1) * BL],
                        wzr_blk(1, jo, kc),
                        hdb[:, kc * BL:(kc + 1) * BL],
                        start=False, stop=(kc == KC - 1),
                    )
            rb = actp.tile([128, FR], bf16, tag="rb")
            nc.scalar.activation(rb[:], pr[:], AF.Sigmoid)

            # ---- z gate matmuls
            pz = pzp.tile([128, FR], f32, tag="pz")
            nc.tensor.matmul(pz[:], ident[:], a0z[:], start=True, stop=False)
            for jo in range(JT):
                for kc in range(KC):
                    nc.tensor.matmul(
                        pz[:, jo * BL:(jo + 1) * BL],
                        wzr_blk(0, jo, kc),
                        hdb[:, kc * BL:(kc + 1) * BL],
                        start=False, stop=(kc == KC - 1),
                    )
            zf = actp.tile([128, FR], f32, tag="zf")
            nc.scalar.activation(zf[:], pz[:], AF.Sigmoid)

            rh = hdp.tile([128, FR], bf16, tag="rh")
            nc.vector.tensor_mul(rh[:], rb[:], hdb[:])

            # ---- projection of h(t-1), first half (fills the sigmoid gap on PE)
            if t > 0:
                pj_prev = pjp.tile([BL, O], f32, tag="pj")
                for kc in (0, 1):
                    nc.tensor.matmul(
                        pj_prev[:],
                        hbf_prev[:, kc * BL:(kc + 1) * BL],
                        wlin[:, kc * O:(kc + 1) * O],
                        start=(kc == 0), stop=False,
                    )

            # ---- candidate gate matmuls
            ph = php.tile([128, FR], f32, tag="ph")
            nc.tensor.matmul(ph[:], ident[:], a0h[:], start=True, stop=False)
            for jo in range(JT):
                for kc in range(KC):
                    nc.tensor.matmul(
                        ph[:, jo * BL:(jo + 1) * BL],
                        wht_blk(jo, kc),
                        rh[:, kc * BL:(kc + 1) * BL],
                        start=False, stop=(kc == KC - 1),
                    )

            # ---- projection of h(t-1), second half (fills the tanh/blend gap)
            if t > 0:
                for kc in (2, 3):
                    nc.tensor.matmul(
                        pj_prev[:],
                        hbf_prev[:, kc * BL:(kc + 1) * BL],
                        wlin[:, kc * O:(kc + 1) * O],
                        start=False, stop=(kc == 3),
                    )
                osb = outp.tile([BL, O], f32, tag="osb")
                nc.vector.tensor_add(osb[:], pj_prev[:], blinb[:])
                nc.sync.dma_start(out_d[t - 1], osb[:])

            hl = actp.tile([128, FR], f32, tag="hl")
            nc.scalar.activation(hl[:], ph[:], AF.Tanh)

            # blend: h = hdf + zf*(hl - hdf)
            dd = actp.tile([128, FR], f32, tag="dd")
            nc.vector.tensor_sub(dd[:], hl[:], hdf[:])
            ee = actp.tile([128, FR], f32, tag="ee")
            nc.vector.tensor_mul(ee[:], dd[:], zf[:])
            nc.vector.tensor_add(h[:], ee[:], hdf[:])

            hbf = outp.tile([128, FR], bf16, tag="hbf")
            nc.vector.tensor_copy(hbf[:], h[:])
            hbf_prev = hbf

        # trailing projection for the last step
        pj_prev = pjp.tile([BL, O], f32, tag="pj")
        for kc in range(KC):
            nc.tensor.matmul(
                pj_prev[:],
                hbf_prev[:, kc * BL:(kc + 1) * BL],
                wlin[:, kc * O:(kc + 1) * O],
                start=(kc == 0), stop=(kc == 3),
            )
        osb = outp.tile([BL, O], f32, tag="osb")
        nc.vector.tensor_add(osb[:], pj_prev[:], blinb[:])
        nc.sync.dma_start(out_d[T - 1], osb[:])

    nc.compile()
    _BUILD_CACHE["nc"] = nc
    return nc


def _host_prep(C, t, Wz, bz, Wr, br, Wh, bh, Wgh, bgh, Wlin, blin):
    """Build per-core input maps (all the precomputed, packed device tensors)."""
    bf = ml_dtypes.bfloat16

    s = Wgh.sum(axis=0)  # (H,)
    t3 = t[:, :, 0]  # (T,B)
    dt = np.concatenate([np.zeros((1, B), np.float32), t3[1:] - t3[:-1]], axis=0)
    # gamma (T,B,H) fp32
    gam = np.exp(-np.maximum(dt[:, :, None] * s[None, None, :] + bgh[None, None, :], 0.0)).astype(np.float32)

    def gate_const(W, b):
        # C @ W_x + colsum(W_m) + b  -> (B,H)
        return C @ W[0:H] + (W[2 * H:3 * H].sum(axis=0) + b)[None, :]

    Az0 = gate_const(Wz, bz).astype(np.float32)
    Ar0 = gate_const(Wr, br).astype(np.float32)
    Ah0 = gate_const(Wh, bh).astype(np.float32)

    Wg = np.stack([Wz[H:2 * H], Wr[H:2 * H]])  # (2,H,H)
    # wzr packed: [k, (kc,g,jo,m)]
    wzr = Wg.reshape(2, KC, 128, JT, 128).transpose(2, 1, 0, 3, 4).reshape(128, KC * 2 * JT * 128)
    wht = Wh[H:2 * H].reshape(KC, 128, JT, 128).transpose(1, 0, 2, 3).reshape(128, KC * JT * 128)
    wlin = Wlin.reshape(KC, 128, O).transpose(1, 0, 2).reshape(128, KC * O)
    wzr = np.ascontiguousarray(wzr, dtype=bf)
    wht = np.ascontiguousarray(wht, dtype=bf)
    wlin = np.ascontiguousarray(wlin, dtype=bf)
    ident = np.eye(128, dtype=bf)

    in_maps = []
    for i in range(NCORES):
        sl = slice(i * BL, (i + 1) * BL)
        gf = gam[:, sl, :]  # (T,BL,H)
        # gam packed: [p, t, kt*BL+b]
        gp = np.ascontiguousarray(gf.reshape(T, BL, KC, 128).transpose(3, 0, 2, 1).reshape(128, T, KC * BL))

        def packA(A):
            return np.ascontiguousarray(
                A[sl].reshape(BL, JT, 128).transpose(2, 1, 0).reshape(128, JT * BL), dtype=bf)

        in_maps.append({
            "gam": gp,
            "wzr": wzr,
            "wht": wht,
            "wlin": wlin,
            "a0z": packA(Az0),
            "a0r": packA(Ar0),
            "a0h": packA(Ah0),
            "ident": ident,
            "blinb": np.ascontiguousarray(np.broadcast_to(blin, (BL, O)), dtype=np.float32),
        })
    return in_maps


def kernel(C, t, mask, Wz, bz, Wr, br, Wh, bh, Wgh, bgh, wgx, bgx, Wlin, blin,
           _trace=False, _trace_kwargs=None):
    C = np.asarray(C, np.float32)
    t = np.asarray(t, np.float32)
    nc = _build_program()
    in_maps = _host_prep(C, t,
                         np.asarray(Wz, np.float32), np.asarray(bz, np.float32),
                         np.asarray(Wr, np.float32), np.asarray(br, np.float32),
                         np.asarray(Wh, np.float32), np.asarray(bh, np.float32),
                         np.asarray(Wgh, np.float32), np.asarray(bgh, np.float32),
                         np.asarray(Wlin, np.float32), np.asarray(blin, np.float32))

    from concourse.bass_utils import run_bass_kernel_spmd
    res = run_bass_kernel_spmd(nc, in_maps, list(range(NCORES)),
                               trace=_trace, **(_trace_kwargs or {}))
    outs = [res.results[i]["out"] for i in range(NCORES)]
    full = np.concatenate(outs, axis=1).astype(np.float32)  # (T,B,O)
    kernel._last_results = res
    return full


# revision 8
# speedup vs baseline: 1.1752x; 1.1752x over previous
"""GRU-D decoder kernel for Trainium2 (8 NeuronCores, data-parallel over batch).

Math (mask == ones everywhere, which the reference hardcodes):
  x_hat = C (constant), d = dt broadcast, gamma_x unused.
  gamma[t,b,j] = exp(-relu(dt[t,b] * colsum(Wgh)[j] + bgh[j]))   (precomputed host-side)
  per step: hdec = gamma_t * h
            z = sigmoid(hdec @ Wz_h + Az0);  r = sigmoid(hdec @ Wr_h + Ar0)
            htl = tanh((r*hdec) @ Wh_h + Ah0)
            h = hdec + z*(htl - hdec)
  out[t] = h_t @ Wlin + blin
  where A?0 = C @ W?_x + colsum(W?_m) + b?  (time-constant, precomputed host-side).

Device layout: everything transposed (H on partitions as 4 tiles of 128,
batch=64 on the free dim), packed as SBUF tiles (128, 4*64) with column
index = kt*64 + b.  Gate matmuls use the weight blocks as stationary
operands and hdec slices as moving operands; outputs land natively in the
same transposed layout, so no transposes are needed anywhere.  The
per-step tail (tanh/blend/decay) is split into two column halves so the
tensor engine can start the next group while the tail of the previous
half is still on Scalar/Vector.
"""

import numpy as np
import ml_dtypes

T, B, H, O = 100, 512, 512, 512
NCORES = 8
BL = B // NCORES  # 64
KC = 4  # contraction chunks of 128
JT = 4  # output j-tiles of 128
FR = JT * BL  # 256
HB = FR // 2  # 128 (half of the free dim; = 2 j-tiles)
GCH = 20  # gamma chunk (steps per DMA)

_BUILD_CACHE = {}


def _build_program():
    if "nc" in _BUILD_CACHE:
        return _BUILD_CACHE["nc"]

    import concourse.tile as tile
    import concourse.mybir as mybir
    from concourse import bacc
    from contextlib import ExitStack

    f32 = mybir.dt.float32
    bf16 = mybir.dt.bfloat16
    AF = mybir.ActivationFunctionType

    nc = bacc.Bacc("TRN2", target_bir_lowering=False, debug=False,
                   num_devices=NCORES)

    gam_d = nc.dram_tensor("gam", [128, T, FR], f32, kind="ExternalInput")
    wzr_d = nc.dram_tensor("wzr", [128, KC * 2 * JT * 128], bf16, kind="ExternalInput")
    wht_d = nc.dram_tensor("wht", [128, KC * JT * 128], bf16, kind="ExternalInput")
    wlin_d = nc.dram_tensor("wlin", [128, KC * O], bf16, kind="ExternalInput")
    a0z_d = nc.dram_tensor("a0z", [128, FR], bf16, kind="ExternalInput")
    a0r_d = nc.dram_tensor("a0r", [128, FR], bf16, kind="ExternalInput")
    a0h_d = nc.dram_tensor("a0h", [128, FR], bf16, kind="ExternalInput")
    ident_d = nc.dram_tensor("ident", [128, 128], bf16, kind="ExternalInput")
    ones_d = nc.dram_tensor("ones64", [1, BL], bf16, kind="ExternalInput")
    blinr_d = nc.dram_tensor("blinr", [1, O], bf16, kind="ExternalInput")
    out_d = nc.dram_tensor("out", [T, BL, O], f32, kind="ExternalOutput")

    with tile.TileContext(nc) as tc, ExitStack() as ctx:
        constp = ctx.enter_context(tc.tile_pool(name="const", bufs=1))
        gpool = ctx.enter_context(tc.tile_pool(name="gam", bufs=2))
        statep = ctx.enter_context(tc.tile_pool(name="state", bufs=1))
        hdp = ctx.enter_context(tc.tile_pool(name="hd", bufs=2))
        actp = ctx.enter_context(tc.tile_pool(name="act", bufs=2))
        pzp = ctx.enter_context(tc.tile_pool(name="pz", bufs=1, space="PSUM"))
        prp = ctx.enter_context(tc.tile_pool(name="pr", bufs=1, space="PSUM"))
        php0 = ctx.enter_context(tc.tile_pool(name="ph0", bufs=1, space="PSUM"))
        php1 = ctx.enter_context(tc.tile_pool(name="ph1", bufs=1, space="PSUM"))
        pjp = ctx.enter_context(tc.tile_pool(name="pj", bufs=2, space="PSUM"))

        wzr = constp.tile([128, KC * 2 * JT * 128], bf16)
        nc.sync.dma_start(wzr[:], wzr_d[:])
        wht = constp.tile([128, KC * JT * 128], bf16)
        nc.sync.dma_start(wht[:], wht_d[:])
        wlin = constp.tile([128, KC * O], bf16)
        nc.sync.dma_start(wlin[:], wlin_d[:])
        a0z = constp.tile([128, FR], bf16)
        nc.sync.dma_start(a0z[:], a0z_d[:])
        a0r = constp.tile([128, FR], bf16)
        nc.sync.dma_start(a0r[:], a0r_d[:])
        a0h = constp.tile([128, FR], bf16)
        nc.sync.dma_start(a0h[:], a0h_d[:])
        ident = constp.tile([128, 128], bf16)
        nc.sync.dma_start(ident[:], ident_d[:])
        ones64 = constp.tile([1, BL], bf16)
        nc.sync.dma_start(ones64[:], ones_d[:])
        blinr = constp.tile([1, O], bf16)
        nc.sync.dma_start(blinr[:], blinr_d[:])

        h = statep.tile([128, FR], f32)
        nc.vector.memset(h[:], 0.0)

        def wzr_blk(g, jo, kc):
            i = ((kc * 2 + g) * JT + jo) * 128
            return wzr[:, i:i + 128]

        def wht_blk(jo, kc):
            i = (kc * JT + jo) * 128
            return wht[:, i:i + 128]

        # gamma chunks, preloaded half a chunk ahead
        chunks = {}

        def ensure_chunk(c):
            if c in chunks or c * GCH >= T:
                return
            t0 = c * GCH
            t1 = min(t0 + GCH, T)
            gt = gpool.tile([128, GCH * FR], f32, tag="gchunk")
            nc.sync.dma_start(gt[:, 0:(t1 - t0) * FR], gam_d[:, t0:t1, :])
            chunks[c] = gt

        def gamma_half(tt, hf):
            c2, o2 = divmod(tt, GCH)
            return chunks[c2][:, o2 * FR + hf * HB: o2 * FR + (hf + 1) * HB]

        ensure_chunk(0)

        # step-0 decayed state is zero
        hdf = hdp.tile([128, FR], f32, tag="hdf")
        nc.vector.memset(hdf[:], 0.0)
        hdb = hdp.tile([128, FR], bf16, tag="hdb")
        nc.vector.memset(hdb[:], 0.0)

        hbf_prev = None

        for t in range(T):
            c, o = divmod(t, GCH)
            if o == GCH // 2:
                ensure_chunk(c + 1)

            # ---- r gate matmuls (kc-major so they start as soon as the
            # first half of hdb is ready)
            pr = prp.tile([128, FR], f32, tag="pr")
            nc.tensor.matmul(pr[:], ident[:], a0r[:], start=True, stop=False)
            for kc in range(KC):
                for jo in range(JT):
                    nc.tensor.matmul(
                        pr[:, jo * BL:(jo + 1) * BL],
                        wzr_blk(1, jo, kc),
                        hdb[:, kc * BL:(kc + 1) * BL],
                        start=False, stop=(kc == KC - 1),
                    )
            rb = actp.tile([128, FR], bf16, tag="rb")
            nc.scalar.activation(rb[:, 0:HB], pr[:, 0:HB], AF.Sigmoid)
            nc.scalar.activation(rb[:, HB:FR], pr[:, HB:FR], AF.Sigmoid)

            # ---- z gate matmuls
            pz = pzp.tile([128, FR], f32, tag="pz")
            nc.tensor.matmul(pz[:], ident[:], a0z[:], start=True, stop=False)
            for kc in range(KC):
                for jo in range(JT):
                    nc.tensor.matmul(
                        pz[:, jo * BL:(jo + 1) * BL],
                        wzr_blk(0, jo, kc),
                        hdb[:, kc * BL:(kc + 1) * BL],
                        start=False, stop=(kc == KC - 1),
                    )
            zf = actp.tile([128, FR], f32, tag="zf")
            nc.scalar.activation(zf[:, 0:HB], pz[:, 0:HB], AF.Sigmoid)
            nc.scalar.activation(zf[:, HB:FR], pz[:, HB:FR], AF.Sigmoid)

            rh = hdp.tile([128, FR], bf16, tag="rh")
            nc.vector.tensor_mul(rh[:, 0:HB], rb[:, 0:HB], hdb[:, 0:HB])
            nc.vector.tensor_mul(rh[:, HB:FR], rb[:, HB:FR], hdb[:, HB:FR])

            # ---- projection of h(t-1) straight into PSUM, DMA'd out from there
            if t > 0:
                pj = pjp.tile([BL, O], f32, tag="pj")
                nc.tensor.matmul(pj[:], ones64[:], blinr[:], start=True, stop=False)
                for kc in range(KC):
                    nc.tensor.matmul(
                        pj[:],
                        hbf_prev[:, kc * BL:(kc + 1) * BL],
                        wlin[:, kc * O:(kc + 1) * O],
                        start=False, stop=(kc == KC - 1),
                    )
                osb = actp.tile([BL, O], f32, tag="osb")
                nc.scalar.copy(osb[:], pj[:])
                nc.sync.dma_start(out_d[t - 1], osb[:])

            # ---- candidate gate matmuls, split into two half-banks so
            # tanh/blend of half 0 overlaps the matmuls of half 1
            ph0 = php0.tile([128, HB], f32, tag="ph0")
            ph1 = php1.tile([128, HB], f32, tag="ph1")
            nc.tensor.matmul(ph0[:], ident[:], a0h[:, 0:HB], start=True, stop=False)
            for kc in range(KC):
                for jo in (0, 1):
                    nc.tensor.matmul(
                        ph0[:, (jo - 0) * BL:(jo - 0 + 1) * BL],
                        wht_blk(jo, kc),
                        rh[:, kc * BL:(kc + 1) * BL],
                        start=False, stop=(kc == KC - 1),
                    )
            nc.tensor.matmul(ph1[:], ident[:], a0h[:, HB:FR], start=True, stop=False)
            for kc in range(KC):
                for jo in (2, 3):
                    nc.tensor.matmul(
                        ph1[:, (jo - 2) * BL:(jo - 2 + 1) * BL],
                        wht_blk(jo, kc),
                        rh[:, kc * BL:(kc + 1) * BL],
                        start=False, stop=(kc == KC - 1),
                    )

            # ---- per-half tail: tanh -> blend -> decay for next step
            hdf_n = hdb_n = None
            if t + 1 < T:
                hdf_n = hdp.tile([128, FR], f32, tag="hdf")
                hdb_n = hdp.tile([128, FR], bf16, tag="hdb")
            for hf, ph in ((0, ph0), (1, ph1)):
                sl = slice(hf * HB, (hf + 1) * HB)
                htl = actp.tile([128, HB], f32, tag=f"htl{hf}")
                nc.scalar.activation(htl[:], ph[:], AF.Tanh)
                dd = actp.tile([128, HB], f32, tag=f"dd{hf}")
                nc.vector.tensor_sub(dd[:], htl[:], hdf[:, sl])
                ee = actp.tile([128, HB], f32, tag=f"ee{hf}")
                nc.vector.tensor_mul(ee[:], dd[:], zf[:, sl])
                nc.vector.tensor_add(h[:, sl], ee[:], hdf[:, sl])
                if t + 1 < T:
                    nc.vector.tensor_mul(hdf_n[:, sl], gamma_half(t + 1, hf), h[:, sl])
                    nc.vector.tensor_copy(hdb_n[:, sl], hdf_n[:, sl])
            if t + 1 < T:
                hdf, hdb = hdf_n, hdb_n

            hbf = actp.tile([128, FR], bf16, tag="hbf")
            nc.vector.tensor_copy(hbf[:], h[:])
            hbf_prev = hbf

        # trailing projection for the last step
        pj = pjp.tile([BL, O], f32, tag="pj")
        nc.tensor.matmul(pj[:], ones64[:], blinr[:], start=True, stop=False)
        for kc in range(KC):
            nc.tensor.matmul(
                pj[:],
                hbf_prev[:, kc * BL:(kc + 1) * BL],
                wlin[:, kc * O:(kc + 1) * O],
                start=False, stop=(kc == KC - 1),
            )
        osb = actp.tile([BL, O], f32, tag="osb")
        nc.scalar.copy(osb[:], pj[:])
        nc.sync.dma_start(out_d[T - 1], osb[:])

    nc.compile()
    _BUILD_CACHE["nc"] = nc
    return nc


def _host_prep(C, t, Wz, bz, Wr, br, Wh, bh, Wgh, bgh, Wlin, blin):
    """Build per-core input maps (all the precomputed, packed device tensors)."""
    bf = ml_dtypes.bfloat16

    s = Wgh.sum(axis=0)  # (H,)
    t3 = t[:, :, 0]  # (T,B)
    dt = np.concatenate([np.zeros((1, B), np.float32), t3[1:] - t3[:-1]], axis=0)
    # gamma (T,B,H) fp32
    gam = np.exp(-np.maximum(dt[:, :, None] * s[None, None, :] + bgh[None, None, :], 0.0)).astype(np.float32)

    def gate_const(W, b):
        # C @ W_x + colsum(W_m) + b  -> (B,H)
        return C @ W[0:H] + (W[2 * H:3 * H].sum(axis=0) + b)[None, :]

    Az0 = gate_const(Wz, bz).astype(np.float32)
    Ar0 = gate_const(Wr, br).astype(np.float32)
    Ah0 = gate_const(Wh, bh).astype(np.float32)

    Wg = np.stack([Wz[H:2 * H], Wr[H:2 * H]])  # (2,H,H)
    # wzr packed: [k, (kc,g,jo,m)]
    wzr = Wg.reshape(2, KC, 128, JT, 128).transpose(2, 1, 0, 3, 4).reshape(128, KC * 2 * JT * 128)
    wht = Wh[H:2 * H].reshape(KC, 128, JT, 128).transpose(1, 0, 2, 3).reshape(128, KC * JT * 128)
    wlin = Wlin.reshape(KC, 128, O).transpose(1, 0, 2).reshape(128, KC * O)
    wzr = np.ascontiguousarray(wzr, dtype=bf)
    wht = np.ascontiguousarray(wht, dtype=bf)
    wlin = np.ascontiguousarray(wlin, dtype=bf)
    ident = np.eye(128, dtype=bf)

    in_maps = []
    for i in range(NCORES):
        sl = slice(i * BL, (i + 1) * BL)
        gf = gam[:, sl, :]  # (T,BL,H)
        # gam packed: [p, t, kt*BL+b]
        gp = np.ascontiguousarray(gf.reshape(T, BL, KC, 128).transpose(3, 0, 2, 1).reshape(128, T, KC * BL))

        def packA(A):
            return np.ascontiguousarray(
                A[sl].reshape(BL, JT, 128).transpose(2, 1, 0).reshape(128, JT * BL), dtype=bf)

        in_maps.append({
            "gam": gp,
            "wzr": wzr,
            "wht": wht,
            "wlin": wlin,
            "a0z": packA(Az0),
            "a0r": packA(Ar0),
            "a0h": packA(Ah0),
            "ident": ident,
            "ones64": np.ones((1, BL), dtype=bf),
            "blinr": np.ascontiguousarray(blin.reshape(1, O), dtype=bf),
        })
    return in_maps


def kernel(C, t, mask, Wz, bz, Wr, br, Wh, bh, Wgh, bgh, wgx, bgx, Wlin, blin,
           _trace=False, _trace_kwargs=None):
    C = np.asarray(C, np.float32)
    t = np.asarray(t, np.float32)
    nc = _build_program()
    in_maps = _host_prep(C, t,
                         np.asarray(Wz, np.float32), np.asarray(bz, np.float32),
                         np.asarray(Wr, np.float32), np.asarray(br, np.float32),
                         np.asarray(Wh, np.float32), np.asarray(bh, np.float32),
                         np.asarray(Wgh, np.float32), np.asarray(bgh, np.float32),
                         np.asarray(Wlin, np.float32), np.asarray(blin, np.float32))

    from concourse.bass_utils import run_bass_kernel_spmd
    res = run_bass_kernel_spmd(nc, in_maps, list(range(NCORES)),
                               trace=_trace, **(_trace_kwargs or {}))
    outs = [res.results[i]["out"] for i in range(NCORES)]
    full = np.concatenate(outs, axis=1).astype(np.float32)  # (T,B,O)
    kernel._last_results = res
    return full
